# revision 7
# baseline (speedup 1.0000x reference)
"""ConvLSTM/GRU TRN2 kernel v5: tunnel-I/O-minimized, on-device transpose,
path-independent NEFF cache key.

vs v2 baseline:
- hout per core is only that core's T/8 time-chunk [TC*B, H] (host concats);
  full h goes to local DRAM, one partition-id-offset DMA slices it out.
  Cuts donated-zero upload + result download 8x (1.07GB -> 134MB tunnel bytes).
- Weights ship sharded (1/8 rows per core), AllGathered on device
  (100MB -> 12.6MB over the axon tunnel).
- x ships in natural [rows, D] layout (host only casts f32->bf16); phase 1
  loads through the XBAR transposing DMA.
- The device-code builder is exec'd from a fixed-name source string on a
  fresh thread, so the BIR's debug table (file/line/traceback per
  instruction) is identical no matter where kernel.py lives or who calls
  it -> the neuronxcc NEFF cache hits across processes and directories.
- Bass module + prepped weights cached across kernel() calls.
"""
import sys
sys.path.insert(0, '/opt/trn_rl_repo')

_BUILD_SRC = r'''
import sys
sys.path.insert(0, '/opt/trn_rl_repo')
from concourse import bass

mybir = bass.mybir
FP32 = mybir.dt.float32
BF16 = mybir.dt.bfloat16
AF = mybir.ActivationFunctionType
ALU = mybir.AluOpType


def build(T=512, NCORES=8):
    B, D, H = 64, 1024, 1024
    TC = T // NCORES
    TB = TC * B
    P1_ITERS = TB // 128
    R_ITERS = T // 2
    KT = 8
    OD = 3 * H
    WX_C = KT * OD          # 24576 wx cols
    WH_C = KT * 2 * H       # 16384 wh cols
    W2_C = KT * H           # 8192  w2 cols
    WALL = WX_C + WH_C + W2_C  # 49152

    nc = bass.Bass(num_devices=NCORES, detect_race_conditions=False,
                   disable_frame_to_traceback=True)

    xt_ext = nc.declare_dram_parameter("xt", [TB, D], BF16, isOutput=False)
    wsl_ext = nc.declare_dram_parameter("wsl", [128 // NCORES, WALL], BF16, isOutput=False)
    hout = nc.declare_dram_parameter("hout", [TB, H], BF16, isOutput=True)

    xc_mine = nc.dram_tensor("xc_mine", [TB, OD], BF16)
    # +128 pad rows so the tail prefetch of the last iteration stays in bounds
    xc_gath = nc.dram_tensor("xc_gath", [T * B + 128, OD], BF16, addr_space="Shared")
    # collectives cannot read IO tensors: stage the weight shard locally first
    wsl_loc = nc.dram_tensor("wsl_loc", [128 // NCORES, WALL], BF16)
    w_gath = nc.dram_tensor("w_gath", [128, WALL], BF16, addr_space="Shared")
    hloc = nc.dram_tensor("hloc", [T * B, H], BF16)

    import contextlib
    with contextlib.ExitStack() as _es:
        wx_sb = _es.enter_context(nc.sbuf_tensor("wx_sb", [128, WX_C], BF16))
        wh_sb = _es.enter_context(nc.sbuf_tensor("wh_sb", [128, WH_C], BF16))
        w2_sb = _es.enter_context(nc.sbuf_tensor("w2_sb", [128, W2_C], BF16))
        xt0 = _es.enter_context(nc.sbuf_tensor("xt0", [128, KT * 128], BF16))
        xt1 = _es.enter_context(nc.sbuf_tensor("xt1", [128, KT * 128], BF16))
        p1st = _es.enter_context(nc.sbuf_tensor("p1st", [128, OD], BF16))
        hT = _es.enter_context(nc.sbuf_tensor("hT", [128, KT * B], BF16))
        rhT = _es.enter_context(nc.sbuf_tensor("rhT", [128, KT * B], BF16))
        xcE = _es.enter_context(nc.sbuf_tensor("xcE", [B, OD], BF16))
        xcO = _es.enter_context(nc.sbuf_tensor("xcO", [B, OD], BF16))
        ri0 = _es.enter_context(nc.sbuf_tensor("ri0", [B, H], BF16))
        ri1 = _es.enter_context(nc.sbuf_tensor("ri1", [B, H], BF16))
        rp0 = _es.enter_context(nc.sbuf_tensor("rp0", [B, H], BF16))
        rp1 = _es.enter_context(nc.sbuf_tensor("rp1", [B, H], BF16))
        rs0 = _es.enter_context(nc.sbuf_tensor("rs0", [B, H], BF16))
        rs1 = _es.enter_context(nc.sbuf_tensor("rs1", [B, H], BF16))
        zi0 = _es.enter_context(nc.sbuf_tensor("zi0", [B, H], BF16))
        zi1 = _es.enter_context(nc.sbuf_tensor("zi1", [B, H], BF16))
        zp0 = _es.enter_context(nc.sbuf_tensor("zp0", [B, H], BF16))
        zp1 = _es.enter_context(nc.sbuf_tensor("zp1", [B, H], BF16))
        zs0 = _es.enter_context(nc.sbuf_tensor("zs0", [B, H], BF16))
        zs1 = _es.enter_context(nc.sbuf_tensor("zs1", [B, H], BF16))
        ui0 = _es.enter_context(nc.sbuf_tensor("ui0", [B, H], BF16))
        ui1 = _es.enter_context(nc.sbuf_tensor("ui1", [B, H], BF16))
        up0 = _es.enter_context(nc.sbuf_tensor("up0", [B, H], BF16))
        up1 = _es.enter_context(nc.sbuf_tensor("up1", [B, H], BF16))
        u2t0 = _es.enter_context(nc.sbuf_tensor("u2t0", [B, H], BF16))
        u2t1 = _es.enter_context(nc.sbuf_tensor("u2t1", [B, H], BF16))
        dt0 = _es.enter_context(nc.sbuf_tensor("dt0", [B, H], BF16))
        dt1 = _es.enter_context(nc.sbuf_tensor("dt1", [B, H], BF16))
        mt0 = _es.enter_context(nc.sbuf_tensor("mt0", [B, H], BF16))
        mt1 = _es.enter_context(nc.sbuf_tensor("mt1", [B, H], BF16))
        rhA = _es.enter_context(nc.sbuf_tensor("rhA", [B, H], BF16))
        rhB = _es.enter_context(nc.sbuf_tensor("rhB", [B, H], BF16))
        hA = _es.enter_context(nc.sbuf_tensor("hA", [B, H], BF16))
        hB = _es.enter_context(nc.sbuf_tensor("hB", [B, H], BF16))
        p1ps = _es.enter_context(nc.psum_tensor("p1ps", [128, OD], FP32))
        w_sem = _es.enter_context(nc.semaphore("w_sem"))
        wg_sem = _es.enter_context(nc.semaphore("wg_sem"))
        xin_sem = _es.enter_context(nc.semaphore("xin_sem"))
        xout_sem = _es.enter_context(nc.semaphore("xout_sem"))
        p1_sem = _es.enter_context(nc.semaphore("p1_sem"))
        p1pe_sem = _es.enter_context(nc.semaphore("p1pe_sem"))
        cc_sem = _es.enter_context(nc.semaphore("cc_sem"))
        pe_sem = _es.enter_context(nc.semaphore("pe_sem"))
        dve_sem = _es.enter_context(nc.semaphore("dve_sem"))
        act_sem = _es.enter_context(nc.semaphore("act_sem"))
        tpo_sem = _es.enter_context(nc.semaphore("tpo_sem"))
        pre_sem = _es.enter_context(nc.semaphore("pre_sem"))
        out_sem = _es.enter_context(nc.semaphore("out_sem"))
        pf_sem = _es.enter_context(nc.semaphore("pf_sem"))
        block = _es.enter_context(nc.Block())
        # recurrence psum views (single col group)
        ccps = p1ps[0:B, 0:2 * H]
        m2ps = p1ps[0:B, 2 * H:3 * H]
        xtb = [xt0, xt1]
        xcb = [xcE, xcO]
        rpb = [rp0, rp1]
        rsb = [rs0, rs1]
        zpb = [zp0, zp1]
        zsb = [zs0, zs1]
        upb = [up0, up1]
        u2b = [u2t0, u2t1]
        dtb = [dt0, dt1]
        mtb = [mt0, mt1]
        rhb = [rhA, rhB]
        hb = [hA, hB]

        # ---------------- SYNC ----------------
        @block.sync
        def _(sy):
            for it in range(P1_ITERS):
                if it >= 2:
                    sy.wait_ge(p1pe_sem, it - 1)   # xt buf reuse
                for k in range(KT):
                    sy.dma_start_transpose(
                        xtb[it % 2][:, 128 * k:128 * (k + 1)],
                        xt_ext[128 * it:128 * (it + 1), 128 * k:128 * (k + 1)],
                    ).then_inc(xin_sem, 16)
                if it > 0:
                    sy.wait_ge(p1_sem, it)
                    sy.dma_start(
                        out=xc_mine[128 * (it - 1):128 * it, :], in_=p1st[:]
                    ).then_inc(xout_sem, 16)
            sy.wait_ge(p1_sem, P1_ITERS)
            sy.dma_start(
                out=xc_mine[128 * (P1_ITERS - 1):128 * P1_ITERS, :], in_=p1st[:]
            ).then_inc(xout_sem, 16)

            sy.wait_ge(pf_sem, 16)
            sy.dma_start(out=xcE[:], in_=xc_gath[0:64, :]).then_inc(pre_sem, 16)
            sy.dma_start(out=xcO[:], in_=xc_gath[64:128, :]).then_inc(pre_sem, 16)

            with (
                sy.register("r_row") as r_row,
                sy.register("r_to") as r_to,
                sy.register("r_d2") as r_d2,
            ):
                sy.reg_mov(r_row, 128)
                sy.reg_mov(r_to, 0)
                sy.reg_mov(r_d2, 0)
                with sy.Fori(0, R_ITERS, 1) as _i:
                    for s in range(2):
                        sy.reg_add(r_d2, r_d2, 2)
                        sy.wait_ge(dve_sem, r_d2)          # rh of step s
                        sy.dma_start_transpose(
                            bass.AP(rhT, 0, [[KT * B, 128], [B, KT], [1, B]]),
                            rhb[s][:],
                        ).then_inc(tpo_sem, 16)
                        sy.reg_add(r_d2, r_d2, 5)
                        sy.wait_ge(dve_sem, r_d2)          # h' of step s
                        sy.dma_start_transpose(
                            bass.AP(hT, 0, [[KT * B, 128], [B, KT], [1, B]]),
                            hb[s][:],
                        ).then_inc(tpo_sem, 16)
                    for s in range(2):
                        off = sy.snap(r_to, donate=False)
                        sy.dma_start(
                            out=hloc[bass.ds(off, B), :], in_=hb[s][:]
                        ).then_inc(out_sem, 16)
                        sy.reg_add(r_to, r_to, B)
                    for s in range(2):
                        off = sy.snap(r_row, donate=False)
                        sy.dma_start(
                            out=xcb[s][:], in_=xc_gath[bass.ds(off, 64), :]
                        ).then_inc(pre_sem, 16)
                        sy.reg_add(r_row, r_row, 64)
                sy.wait_ge(out_sem, 16 * T)
            # slice my T/8 chunk of h out to the (small) external output
            pid = sy.partition_id()
            with sy.register("r_fo") as r_fo:
                sy.reg_mul(r_fo, pid, TB)
                foff = sy.snap(r_fo, donate=False, min_val=0,
                               max_val=(NCORES - 1) * TB)
                sy.dma_start(
                    out=hout[:, :], in_=hloc[bass.ds(foff, TB), :]
                ).then_inc(out_sem, 16)
                sy.wait_ge(out_sem, 16 * T + 16)

        # ---------------- PE ----------------
        @block.tensor
        def _(pe):
            pe.wait_ge(w_sem, 48)
            for it in range(P1_ITERS):
                pe.wait_ge(xin_sem, 128 * (it + 1))
                if it > 0:
                    pe.wait_ge(p1_sem, it)
                for k in range(KT):
                    for n in range(OD // 512):
                        mmi = pe.matmul(
                            p1ps[:, 512 * n:512 * (n + 1)],
                            xtb[it % 2][:, 128 * k:128 * (k + 1)],
                            wx_sb[:, OD * k + 512 * n: OD * k + 512 * (n + 1)],
                            start=(k == 0), stop=(k == KT - 1),
                        )
                mmi.then_inc(p1pe_sem, 1)
            pe.wait_ge(p1_sem, P1_ITERS)

            with pe.register("p_tp") as p_tp:
                pe.reg_mov(p_tp, 32)
                with pe.Fori(0, R_ITERS, 1) as _i:
                    for s in range(2):
                        pe.wait_ge(tpo_sem, p_tp)      # h'.T(t-1)
                        # mm1, r-half (cc cols H:2H, n-tiles 2,3) first so the
                        # sigma(r)/rh/transpose chain overlaps the z-half.
                        for half in range(2):          # 0: r (n=2,3), 1: z (n=0,1)
                            for n in ((2, 3), (0, 1))[half]:
                                for k in range(KT):
                                    mm1i = pe.matmul(
                                        ccps[:, 512 * n:512 * (n + 1)],
                                        hT[:, B * k:B * (k + 1)],
                                        wh_sb[:, 2 * H * k + 512 * n:
                                              2 * H * k + 512 * (n + 1)],
                                        start=(k == 0), stop=(k == KT - 1),
                                    )
                            mm1i.then_inc(pe_sem, 1)
                        pe.reg_add(p_tp, p_tp, 16)
                        pe.wait_ge(tpo_sem, p_tp)      # rh.T(t)
                        for n in range(2):
                            for k in range(KT):
                                mm2i = pe.matmul(
                                    m2ps[:, 512 * n:512 * (n + 1)],
                                    rhT[:, B * k:B * (k + 1)],
                                    w2_sb[:, H * k + 512 * n:
                                          H * k + 512 * (n + 1)],
                                    start=(k == 0), stop=(k == KT - 1),
                                )
                        mm2i.then_inc(pe_sem, 1)
                        pe.reg_add(p_tp, p_tp, 16)

        # ---------------- DVE ----------------
        @block.vector
        def _(v):
            for it in range(P1_ITERS):
                v.wait_ge(p1pe_sem, it + 1)
                v.tensor_copy(p1st[:], p1ps[:]).then_inc(p1_sem, 1)
            v.wait_ge(pf_sem, 16)
            v.memset(hT[:], 1e-9).then_inc(tpo_sem, 16)
            v.memset(hB[:], 1e-9).then_inc(tpo_sem, 16)
            with (
                v.register("v_pe") as v_pe,
                v.register("v_ac") as v_ac,
                v.register("v_ou") as v_ou,
                v.register("v_pr") as v_pr,
            ):
                v.reg_mov(v_pe, 1)
                v.reg_mov(v_ac, 1)
                v.reg_mov(v_ou, 0)
                v.reg_mov(v_pr, 32)
                with v.Fori(0, R_ITERS, 1) as _i:
                    v.wait_ge(pre_sem, v_pr)
                    for s in range(2):
                        hprev = hb[1 - s]
                        # r chain
                        v.wait_ge(pe_sem, v_pe)                      # mm1 r half
                        v.tensor_add(rpb[s][:], ccps[:, H:2 * H], xcb[s][:, H:2 * H]).then_inc(dve_sem, 1)
                        v.wait_ge(act_sem, v_ac)                     # sigma(r)
                        v.tensor_mul(rhb[s][:], rsb[s][:], hprev[:]).then_inc(dve_sem, 1)
                        # z
                        v.reg_add(v_pe, v_pe, 1)
                        v.wait_ge(pe_sem, v_pe)                      # mm1 z half
                        v.tensor_add(zpb[s][:], ccps[:, 0:H], xcb[s][:, 0:H]).then_inc(dve_sem, 1)
                        # u chain
                        v.reg_add(v_pe, v_pe, 1)
                        v.wait_ge(pe_sem, v_pe)                      # mm2
                        v.tensor_add(upb[s][:], m2ps, xcb[s][:, 2 * H:3 * H]).then_inc(dve_sem, 1)
                        v.reg_add(v_ac, v_ac, 2)
                        v.wait_ge(act_sem, v_ac)                     # tanh(u)
                        v.tensor_sub(dtb[s][:], u2b[s][:], hprev[:]).then_inc(dve_sem, 1)
                        v.tensor_mul(mtb[s][:], zsb[s][:], dtb[s][:]).then_inc(dve_sem, 1)
                        v.wait_ge(out_sem, v_ou)
                        v.tensor_add(hb[s][:], hprev[:], mtb[s][:]).then_inc(dve_sem, 1)
                        v.reg_add(v_pe, v_pe, 1)
                        v.reg_add(v_ac, v_ac, 1)
                    v.reg_add(v_ou, v_ou, 32)
                    v.reg_add(v_pr, v_pr, 32)

        # ---------------- ACT ----------------
        @block.scalar
        def _(a):
            a.wait_ge(pf_sem, 16)
            with a.register("a_dv") as a_dv:
                a.reg_mov(a_dv, 1)
                with a.Fori(0, R_ITERS, 1) as _i:
                    for s in range(2):
                        a.wait_ge(dve_sem, a_dv)                     # rp
                        a.activation(rsb[s][:], rpb[s][:], AF.Sigmoid).then_inc(act_sem, 1)
                        a.reg_add(a_dv, a_dv, 2)
                        a.wait_ge(dve_sem, a_dv)                     # zp
                        a.activation(zsb[s][:], zpb[s][:], AF.Sigmoid).then_inc(act_sem, 1)
                        a.reg_add(a_dv, a_dv, 1)
                        a.wait_ge(dve_sem, a_dv)                     # up
                        a.activation(u2b[s][:], upb[s][:], AF.Tanh).then_inc(act_sem, 1)
                        a.reg_add(a_dv, a_dv, 4)

        # ---------------- GPSIMD ----------------
        @block.gpsimd
        def _(gp):
            # weights: AllGather the 1/8-row shards, then stage to SBUF
            gp.dma_start(out=wsl_loc[:], in_=wsl_ext[:]).then_inc(wg_sem, 16)
            gp.wait_ge(wg_sem, 16)
            gp.collective_compute(
                "AllGather", ALU.bypass,
                ins=[wsl_loc[:]], outs=[w_gath[:]],
                replica_groups=[list(range(NCORES))],
            ).then_inc(wg_sem, 1)
            gp.wait_ge(wg_sem, 17)
            gp.dma_start(out=wx_sb[:], in_=w_gath[:, 0:WX_C]).then_inc(w_sem, 16)
            gp.dma_start(out=wh_sb[:], in_=w_gath[:, WX_C:WX_C + WH_C]).then_inc(w_sem, 16)
            gp.dma_start(out=w2_sb[:], in_=w_gath[:, WX_C + WH_C:WALL]).then_inc(w_sem, 16)
            gp.wait_ge(xout_sem, 16 * P1_ITERS)
            gp.collective_compute(
                "AllGather", ALU.bypass,
                ins=[xc_mine[:]], outs=[xc_gath[0:T * 64, :]],
                replica_groups=[list(range(NCORES))],
            ).then_inc(cc_sem, 1)
            gp.wait_ge(cc_sem, 1)
            gp.dma_start(
                out=xc_gath[T * 64:T * 64 + 128, :], in_=xc_gath[0:128, :]
            ).then_inc(pf_sem, 16)

    return nc


def _thread_build(out, T, NCORES):
    try:
        out["nc"] = build(T=T, NCORES=NCORES)
    except BaseException as e:
        out["err"] = e
'''


def _build_nc(T, NCORES):
    """Build the Bass module with a caller/path-independent debug table.

    The builder source is exec'd under a fixed pseudo-filename and run on a
    fresh thread, so the per-instruction debug info (filename/lineno/
    traceback) that lands in the BIR json is byte-identical regardless of
    where this file lives or what the calling stack looks like. That keeps
    the neuronxcc NEFF cache key stable across processes and directories.
    """
    import threading
    ns: dict = {}
    exec(compile(_BUILD_SRC, "<convlstm_bass_v5>", "exec"), ns)
    out: dict = {}
    th = threading.Thread(target=ns["_thread_build"], args=(out, T, NCORES))
    th.start()
    th.join()
    if "err" in out:
        raise out["err"]
    return out["nc"]


_CACHE: dict = {}
_T, _B, _D, _H, _NC = 512, 64, 1024, 1024, 8


def _install_cc_cache():
    """Memoize the BIR->NEFF compile on HLO content.

    bass2jax recompiles the (unchanged) BIR on every jit trace; walrus takes
    ~0.4s per call even warm. The compile is a pure function of the HLO bytes
    (which embed the zstd'd BIR), so cache it process-locally."""
    from concourse import bass2jax
    if getattr(bass2jax, "_convlstm_cc_cache", False):
        return
    import hashlib
    base_hook = bass2jax.neuronx_cc_hook
    cc_cache: dict = {}

    def _cached_hook(code, code_format, platform_version, file_prefix):
        key = (hashlib.blake2b(bytes(code), digest_size=16).digest(),
               bytes(code_format))
        r = cc_cache.get(key)
        if r is None:
            r = base_hook(code, code_format, platform_version, file_prefix)
            if len(cc_cache) < 8:
                cc_cache[key] = r
        return r

    bass2jax.neuronx_cc_hook = _cached_hook
    bass2jax._convlstm_cc_cache = True


def _prep_w(Wfc, Wfc2):
    """Host-side weight prep -> per-core [16, 49152] bf16 row shards."""
    import numpy as np
    import ml_dtypes
    D, H, NC = _D, _H, _NC
    bf = ml_dtypes.bfloat16
    Wx = np.concatenate([Wfc[:H, :D].T, Wfc[H:, :D].T, Wfc2[:, :D].T], axis=1)   # [D, 3H]
    Wh = np.concatenate([Wfc[:H, D:].T, Wfc[H:, D:].T], axis=1)                  # [H, 2H]
    W2 = Wfc2[:, D:].T                                                           # [H, H]

    def kmaj(w):  # [K, M] -> [128, (K/128)*M]
        K, M = w.shape
        return np.ascontiguousarray(
            w.reshape(K // 128, 128, M).transpose(1, 0, 2).reshape(128, (K // 128) * M)
        ).astype(bf)

    w_all = np.concatenate([kmaj(Wx), kmaj(Wh), kmaj(W2)], axis=1)  # [128, 49152]
    rpc = 128 // NC
    return [np.ascontiguousarray(w_all[rpc * c:rpc * (c + 1)]) for c in range(NC)]


def _prep_x(x):
    """x [T,B,D] f32 -> per-core natural-layout bf16 chunks [TC*B, D]."""
    import ml_dtypes
    bf = ml_dtypes.bfloat16
    T, B, D = x.shape
    TC = T // _NC
    return [
        x[c * TC:(c + 1) * TC].reshape(TC * B, D).astype(bf) for c in range(_NC)
    ]


def _wfp(Wfc, Wfc2):
    """Cheap weight fingerprint: shapes + sums + sum-of-squares (memory-bound,
    ~10ms; detects any realistic weight change)."""
    import numpy as np
    a, b = Wfc.ravel(), Wfc2.ravel()
    return (
        Wfc.shape, Wfc2.shape,
        float(a.sum()), float(np.einsum("i,i->", a, a)),
        float(b.sum()), float(np.einsum("i,i->", b, b)),
    )


def kernel(**inputs):
    """Full-input kernel: x [512,64,1024] f32 -> h_seq [512,64,1024] f32."""
    import numpy as np
    x = np.asarray(inputs["x"], dtype=np.float32)
    Wfc = np.asarray(inputs["Wfc"], dtype=np.float32)
    Wfc2 = np.asarray(inputs["Wfc2"], dtype=np.float32)

    if "nc" not in _CACHE:
        _CACHE["nc"] = _build_nc(_T, _NC)
    _install_cc_cache()
    wkey = _wfp(Wfc, Wfc2)
    if _CACHE.get("wkey") != wkey:
        _CACHE["w"] = _prep_w(Wfc, Wfc2)
        _CACHE["wkey"] = wkey

    xs = _prep_x(x)
    in_maps = [{"xt": xs[c], "wsl": _CACHE["w"][c]} for c in range(_NC)]

    from concourse.bass_utils import run_bass_kernel_spmd
    try:
        res = run_bass_kernel_spmd(_CACHE["nc"], in_maps, list(range(_NC)))
    except Exception:
        # transient device/tunnel hiccups happen; one retry
        import time as _time
        _time.sleep(2.0)
        res = run_bass_kernel_spmd(_CACHE["nc"], in_maps, list(range(_NC)))
    out = np.empty((_T * _B, _H), dtype=np.float32)
    TB = _T * _B // _NC
    for c in range(_NC):
        out[c * TB:(c + 1) * TB] = res.results[c]["hout"]
    kernel.last_exec_time_ns = getattr(res, "exec_time_ns", None)
    return out.reshape(_T, _B, _H)


# revision 8
# speedup vs baseline: 1.1615x; 1.1615x over previous
"""ConvLSTM/GRU TRN2 kernel v5: tunnel-I/O-minimized, on-device transpose,
path-independent NEFF cache key.

vs v2 baseline:
- hout per core is only that core's T/8 time-chunk [TC*B, H] (host concats);
  full h goes to local DRAM, one partition-id-offset DMA slices it out.
  Cuts donated-zero upload + result download 8x (1.07GB -> 134MB tunnel bytes).
- Weights ship sharded (1/8 rows per core), AllGathered on device
  (100MB -> 12.6MB over the axon tunnel).
- x ships in natural [rows, D] layout (host only casts f32->bf16); phase 1
  loads through the XBAR transposing DMA.
- The device-code builder is exec'd from a fixed-name source string on a
  fresh thread, so the BIR's debug table (file/line/traceback per
  instruction) is identical no matter where kernel.py lives or who calls
  it -> the neuronxcc NEFF cache hits across processes and directories.
- Bass module + prepped weights cached across kernel() calls.
"""
import sys
sys.path.insert(0, '/opt/trn_rl_repo')

_BUILD_SRC = r'''
import sys
sys.path.insert(0, '/opt/trn_rl_repo')
from concourse import bass

mybir = bass.mybir
FP32 = mybir.dt.float32
BF16 = mybir.dt.bfloat16
AF = mybir.ActivationFunctionType
ALU = mybir.AluOpType


def build(T=512, NCORES=8):
    B, D, H = 64, 1024, 1024
    TC = T // NCORES
    TB = TC * B
    P1_ITERS = TB // 128
    R_ITERS = T // 2
    KT = 8
    OD = 3 * H
    WX_C = KT * OD          # 24576 wx cols
    WH_C = KT * 2 * H       # 16384 wh cols
    W2_C = KT * H           # 8192  w2 cols
    WALL = WX_C + WH_C + W2_C  # 49152

    nc = bass.Bass(num_devices=NCORES, detect_race_conditions=False,
                   disable_frame_to_traceback=True)

    xt_ext = nc.declare_dram_parameter("xt", [TB, D], BF16, isOutput=False)
    wsl_ext = nc.declare_dram_parameter("wsl", [128 // NCORES, WALL], BF16, isOutput=False)
    hout = nc.declare_dram_parameter("hout", [TB, H], BF16, isOutput=True)

    xc_mine = nc.dram_tensor("xc_mine", [TB, OD], BF16)
    # +128 pad rows so the tail prefetch of the last iteration stays in bounds
    xc_gath = nc.dram_tensor("xc_gath", [T * B + 128, OD], BF16, addr_space="Shared")
    # collectives cannot read IO tensors: stage the weight shard locally first
    wsl_loc = nc.dram_tensor("wsl_loc", [128 // NCORES, WALL], BF16)
    w_gath = nc.dram_tensor("w_gath", [128, WALL], BF16, addr_space="Shared")
    hloc = nc.dram_tensor("hloc", [T * B, H], BF16)

    import contextlib
    with contextlib.ExitStack() as _es:
        wx_sb = _es.enter_context(nc.sbuf_tensor("wx_sb", [128, WX_C], BF16))
        wh_sb = _es.enter_context(nc.sbuf_tensor("wh_sb", [128, WH_C], BF16))
        w2_sb = _es.enter_context(nc.sbuf_tensor("w2_sb", [128, W2_C], BF16))
        xt0 = _es.enter_context(nc.sbuf_tensor("xt0", [128, KT * 128], BF16))
        xt1 = _es.enter_context(nc.sbuf_tensor("xt1", [128, KT * 128], BF16))
        p1st = _es.enter_context(nc.sbuf_tensor("p1st", [128, OD], BF16))
        hT = _es.enter_context(nc.sbuf_tensor("hT", [128, KT * B], BF16))
        rhT = _es.enter_context(nc.sbuf_tensor("rhT", [128, KT * B], BF16))
        xcE = _es.enter_context(nc.sbuf_tensor("xcE", [B, OD], BF16))
        xcO = _es.enter_context(nc.sbuf_tensor("xcO", [B, OD], BF16))
        ri0 = _es.enter_context(nc.sbuf_tensor("ri0", [B, H], BF16))
        ri1 = _es.enter_context(nc.sbuf_tensor("ri1", [B, H], BF16))
        rp0 = _es.enter_context(nc.sbuf_tensor("rp0", [B, H], BF16))
        rp1 = _es.enter_context(nc.sbuf_tensor("rp1", [B, H], BF16))
        rs0 = _es.enter_context(nc.sbuf_tensor("rs0", [B, H], BF16))
        rs1 = _es.enter_context(nc.sbuf_tensor("rs1", [B, H], BF16))
        zi0 = _es.enter_context(nc.sbuf_tensor("zi0", [B, H], BF16))
        zi1 = _es.enter_context(nc.sbuf_tensor("zi1", [B, H], BF16))
        zp0 = _es.enter_context(nc.sbuf_tensor("zp0", [B, H], BF16))
        zp1 = _es.enter_context(nc.sbuf_tensor("zp1", [B, H], BF16))
        zs0 = _es.enter_context(nc.sbuf_tensor("zs0", [B, H], BF16))
        zs1 = _es.enter_context(nc.sbuf_tensor("zs1", [B, H], BF16))
        ui0 = _es.enter_context(nc.sbuf_tensor("ui0", [B, H], BF16))
        ui1 = _es.enter_context(nc.sbuf_tensor("ui1", [B, H], BF16))
        up0 = _es.enter_context(nc.sbuf_tensor("up0", [B, H], BF16))
        up1 = _es.enter_context(nc.sbuf_tensor("up1", [B, H], BF16))
        u2t0 = _es.enter_context(nc.sbuf_tensor("u2t0", [B, H], BF16))
        u2t1 = _es.enter_context(nc.sbuf_tensor("u2t1", [B, H], BF16))
        dt0 = _es.enter_context(nc.sbuf_tensor("dt0", [B, H], BF16))
        dt1 = _es.enter_context(nc.sbuf_tensor("dt1", [B, H], BF16))
        mt0 = _es.enter_context(nc.sbuf_tensor("mt0", [B, H], BF16))
        mt1 = _es.enter_context(nc.sbuf_tensor("mt1", [B, H], BF16))
        rhA = _es.enter_context(nc.sbuf_tensor("rhA", [B, H], BF16))
        rhB = _es.enter_context(nc.sbuf_tensor("rhB", [B, H], BF16))
        hA = _es.enter_context(nc.sbuf_tensor("hA", [B, H], BF16))
        hB = _es.enter_context(nc.sbuf_tensor("hB", [B, H], BF16))
        p1ps = _es.enter_context(nc.psum_tensor("p1ps", [128, OD], FP32))
        w_sem = _es.enter_context(nc.semaphore("w_sem"))
        wg_sem = _es.enter_context(nc.semaphore("wg_sem"))
        xin_sem = _es.enter_context(nc.semaphore("xin_sem"))
        xout_sem = _es.enter_context(nc.semaphore("xout_sem"))
        p1_sem = _es.enter_context(nc.semaphore("p1_sem"))
        p1pe_sem = _es.enter_context(nc.semaphore("p1pe_sem"))
        cc_sem = _es.enter_context(nc.semaphore("cc_sem"))
        pe_sem = _es.enter_context(nc.semaphore("pe_sem"))
        dve_sem = _es.enter_context(nc.semaphore("dve_sem"))
        act_sem = _es.enter_context(nc.semaphore("act_sem"))
        tpo_sem = _es.enter_context(nc.semaphore("tpo_sem"))
        pre_sem = _es.enter_context(nc.semaphore("pre_sem"))
        out_sem = _es.enter_context(nc.semaphore("out_sem"))
        pf_sem = _es.enter_context(nc.semaphore("pf_sem"))
        block = _es.enter_context(nc.Block())
        # recurrence psum views (single col group)
        ccps = p1ps[0:B, 0:2 * H]
        m2ps = p1ps[0:B, 2 * H:3 * H]
        xtb = [xt0, xt1]
        xcb = [xcE, xcO]
        rpb = [rp0, rp1]
        rsb = [rs0, rs1]
        zpb = [zp0, zp1]
        zsb = [zs0, zs1]
        upb = [up0, up1]
        u2b = [u2t0, u2t1]
        dtb = [dt0, dt1]
        mtb = [mt0, mt1]
        rhb = [rhA, rhB]
        hb = [hA, hB]

        # ---------------- SYNC ----------------
        @block.sync
        def _(sy):
            for it in range(P1_ITERS):
                if it >= 2:
                    sy.wait_ge(p1pe_sem, it - 1)   # xt buf reuse
                for k in range(KT):
                    sy.dma_start_transpose(
                        xtb[it % 2][:, 128 * k:128 * (k + 1)],
                        xt_ext[128 * it:128 * (it + 1), 128 * k:128 * (k + 1)],
                    ).then_inc(xin_sem, 16)
                if it > 0:
                    sy.wait_ge(p1_sem, it)
                    sy.dma_start(
                        out=xc_mine[128 * (it - 1):128 * it, :], in_=p1st[:]
                    ).then_inc(xout_sem, 16)
            sy.wait_ge(p1_sem, P1_ITERS)
            sy.dma_start(
                out=xc_mine[128 * (P1_ITERS - 1):128 * P1_ITERS, :], in_=p1st[:]
            ).then_inc(xout_sem, 16)

            sy.wait_ge(pf_sem, 16)
            sy.dma_start(out=xcE[:], in_=xc_gath[0:64, :]).then_inc(pre_sem, 16)
            sy.dma_start(out=xcO[:], in_=xc_gath[64:128, :]).then_inc(pre_sem, 16)

            with (
                sy.register("r_row") as r_row,
                sy.register("r_to") as r_to,
                sy.register("r_d2") as r_d2,
            ):
                sy.reg_mov(r_row, 128)
                sy.reg_mov(r_to, 0)
                sy.reg_mov(r_d2, 0)
                with sy.Fori(0, R_ITERS, 1) as _i:
                    for s in range(2):
                        sy.reg_add(r_d2, r_d2, 2)
                        sy.wait_ge(dve_sem, r_d2)          # rh of step s
                        sy.dma_start_transpose(
                            bass.AP(rhT, 0, [[KT * B, 128], [B, KT], [1, B]]),
                            rhb[s][:],
                        ).then_inc(tpo_sem, 16)
                        sy.reg_add(r_d2, r_d2, 5)
                        sy.wait_ge(dve_sem, r_d2)          # h' of step s
                        sy.dma_start_transpose(
                            bass.AP(hT, 0, [[KT * B, 128], [B, KT], [1, B]]),
                            hb[s][:],
                        ).then_inc(tpo_sem, 16)
                    for s in range(2):
                        off = sy.snap(r_to, donate=False)
                        sy.dma_start(
                            out=hloc[bass.ds(off, B), :], in_=hb[s][:]
                        ).then_inc(out_sem, 16)
                        sy.reg_add(r_to, r_to, B)
                    for s in range(2):
                        off = sy.snap(r_row, donate=False)
                        sy.dma_start(
                            out=xcb[s][:], in_=xc_gath[bass.ds(off, 64), :]
                        ).then_inc(pre_sem, 16)
                        sy.reg_add(r_row, r_row, 64)
                sy.wait_ge(out_sem, 16 * T)
            # slice my T/8 chunk of h out to the (small) external output
            pid = sy.partition_id()
            with sy.register("r_fo") as r_fo:
                sy.reg_mul(r_fo, pid, TB)
                foff = sy.snap(r_fo, donate=False, min_val=0,
                               max_val=(NCORES - 1) * TB)
                sy.dma_start(
                    out=hout[:, :], in_=hloc[bass.ds(foff, TB), :]
                ).then_inc(out_sem, 16)
                sy.wait_ge(out_sem, 16 * T + 16)

        # ---------------- PE ----------------
        @block.tensor
        def _(pe):
            pe.wait_ge(w_sem, 48)
            for it in range(P1_ITERS):
                pe.wait_ge(xin_sem, 128 * (it + 1))
                if it > 0:
                    pe.wait_ge(p1_sem, it)
                for k in range(KT):
                    for n in range(OD // 512):
                        mmi = pe.matmul(
                            p1ps[:, 512 * n:512 * (n + 1)],
                            xtb[it % 2][:, 128 * k:128 * (k + 1)],
                            wx_sb[:, OD * k + 512 * n: OD * k + 512 * (n + 1)],
                            start=(k == 0), stop=(k == KT - 1),
                        )
                mmi.then_inc(p1pe_sem, 1)
            pe.wait_ge(p1_sem, P1_ITERS)

            with pe.register("p_tp") as p_tp:
                pe.reg_mov(p_tp, 32)
                with pe.Fori(0, R_ITERS, 1) as _i:
                    for s in range(2):
                        pe.wait_ge(tpo_sem, p_tp)      # h'.T(t-1)
                        # mm1, r-half (cc cols H:2H, n-tiles 2,3) first so the
                        # sigma(r)/rh/transpose chain overlaps the z-half.
                        for half in range(2):          # 0: r (n=2,3), 1: z (n=0,1)
                            for n in ((2, 3), (0, 1))[half]:
                                for k in range(KT):
                                    mm1i = pe.matmul(
                                        ccps[:, 512 * n:512 * (n + 1)],
                                        hT[:, B * k:B * (k + 1)],
                                        wh_sb[:, 2 * H * k + 512 * n:
                                              2 * H * k + 512 * (n + 1)],
                                        start=(k == 0), stop=(k == KT - 1),
                                    )
                            mm1i.then_inc(pe_sem, 1)
                        pe.reg_add(p_tp, p_tp, 16)
                        pe.wait_ge(tpo_sem, p_tp)      # rh.T(t)
                        for n in range(2):
                            for k in range(KT):
                                mm2i = pe.matmul(
                                    m2ps[:, 512 * n:512 * (n + 1)],
                                    rhT[:, B * k:B * (k + 1)],
                                    w2_sb[:, H * k + 512 * n:
                                          H * k + 512 * (n + 1)],
                                    start=(k == 0), stop=(k == KT - 1),
                                )
                        mm2i.then_inc(pe_sem, 1)
                        pe.reg_add(p_tp, p_tp, 16)

        # ---------------- DVE ----------------
        @block.vector
        def _(v):
            for it in range(P1_ITERS):
                v.wait_ge(p1pe_sem, it + 1)
                v.tensor_copy(p1st[:], p1ps[:]).then_inc(p1_sem, 1)
            v.wait_ge(pf_sem, 16)
            v.memset(hT[:], 1e-9).then_inc(tpo_sem, 16)
            v.memset(hB[:], 1e-9).then_inc(tpo_sem, 16)
            with (
                v.register("v_pe") as v_pe,
                v.register("v_ac") as v_ac,
                v.register("v_ou") as v_ou,
                v.register("v_pr") as v_pr,
            ):
                v.reg_mov(v_pe, 1)
                v.reg_mov(v_ac, 1)
                v.reg_mov(v_ou, 0)
                v.reg_mov(v_pr, 32)
                with v.Fori(0, R_ITERS, 1) as _i:
                    v.wait_ge(pre_sem, v_pr)
                    for s in range(2):
                        hprev = hb[1 - s]
                        # r chain
                        v.wait_ge(pe_sem, v_pe)                      # mm1 r half
                        v.tensor_add(rpb[s][:], ccps[:, H:2 * H], xcb[s][:, H:2 * H]).then_inc(dve_sem, 1)
                        v.wait_ge(act_sem, v_ac)                     # sigma(r)
                        v.tensor_mul(rhb[s][:], rsb[s][:], hprev[:]).then_inc(dve_sem, 1)
                        # z
                        v.reg_add(v_pe, v_pe, 1)
                        v.wait_ge(pe_sem, v_pe)                      # mm1 z half
                        v.tensor_add(zpb[s][:], ccps[:, 0:H], xcb[s][:, 0:H]).then_inc(dve_sem, 1)
                        # u chain
                        v.reg_add(v_pe, v_pe, 1)
                        v.wait_ge(pe_sem, v_pe)                      # mm2
                        v.tensor_add(upb[s][:], m2ps, xcb[s][:, 2 * H:3 * H]).then_inc(dve_sem, 1)
                        v.reg_add(v_ac, v_ac, 2)
                        v.wait_ge(act_sem, v_ac)                     # tanh(u)
                        v.tensor_sub(dtb[s][:], u2b[s][:], hprev[:]).then_inc(dve_sem, 1)
                        v.tensor_mul(mtb[s][:], zsb[s][:], dtb[s][:]).then_inc(dve_sem, 1)
                        v.wait_ge(out_sem, v_ou)
                        v.tensor_add(hb[s][:], hprev[:], mtb[s][:]).then_inc(dve_sem, 1)
                        v.reg_add(v_pe, v_pe, 1)
                        v.reg_add(v_ac, v_ac, 1)
                    v.reg_add(v_ou, v_ou, 32)
                    v.reg_add(v_pr, v_pr, 32)

        # ---------------- ACT ----------------
        @block.scalar
        def _(a):
            a.wait_ge(pf_sem, 16)
            with a.register("a_dv") as a_dv:
                a.reg_mov(a_dv, 1)
                with a.Fori(0, R_ITERS, 1) as _i:
                    for s in range(2):
                        a.wait_ge(dve_sem, a_dv)                     # rp
                        a.activation(rsb[s][:], rpb[s][:], AF.Sigmoid).then_inc(act_sem, 1)
                        a.reg_add(a_dv, a_dv, 2)
                        a.wait_ge(dve_sem, a_dv)                     # zp
                        a.activation(zsb[s][:], zpb[s][:], AF.Sigmoid).then_inc(act_sem, 1)
                        a.reg_add(a_dv, a_dv, 1)
                        a.wait_ge(dve_sem, a_dv)                     # up
                        a.activation(u2b[s][:], upb[s][:], AF.Tanh).then_inc(act_sem, 1)
                        a.reg_add(a_dv, a_dv, 4)

        # ---------------- GPSIMD ----------------
        @block.gpsimd
        def _(gp):
            # weights: AllGather the 1/8-row shards, then stage to SBUF
            gp.dma_start(out=wsl_loc[:], in_=wsl_ext[:]).then_inc(wg_sem, 16)
            gp.wait_ge(wg_sem, 16)
            gp.collective_compute(
                "AllGather", ALU.bypass,
                ins=[wsl_loc[:]], outs=[w_gath[:]],
                replica_groups=[list(range(NCORES))],
            ).then_inc(wg_sem, 1)
            gp.wait_ge(wg_sem, 17)
            gp.dma_start(out=wx_sb[:], in_=w_gath[:, 0:WX_C]).then_inc(w_sem, 16)
            gp.dma_start(out=wh_sb[:], in_=w_gath[:, WX_C:WX_C + WH_C]).then_inc(w_sem, 16)
            gp.dma_start(out=w2_sb[:], in_=w_gath[:, WX_C + WH_C:WALL]).then_inc(w_sem, 16)
            gp.wait_ge(xout_sem, 16 * P1_ITERS)
            gp.collective_compute(
                "AllGather", ALU.bypass,
                ins=[xc_mine[:]], outs=[xc_gath[0:T * 64, :]],
                replica_groups=[list(range(NCORES))],
            ).then_inc(cc_sem, 1)
            gp.wait_ge(cc_sem, 1)
            gp.dma_start(
                out=xc_gath[T * 64:T * 64 + 128, :], in_=xc_gath[0:128, :]
            ).then_inc(pf_sem, 16)

    return nc


def _thread_build(out, T, NCORES):
    try:
        out["nc"] = build(T=T, NCORES=NCORES)
    except BaseException as e:
        out["err"] = e
'''


def _build_nc(T, NCORES):
    """Build the Bass module with a caller/path-independent debug table.

    The builder source is exec'd under a fixed pseudo-filename and run on a
    fresh thread, so the per-instruction debug info (filename/lineno/
    traceback) that lands in the BIR json is byte-identical regardless of
    where this file lives or what the calling stack looks like. That keeps
    the neuronxcc NEFF cache key stable across processes and directories.
    """
    import threading
    ns: dict = {}
    exec(compile(_BUILD_SRC, "<convlstm_bass_v5>", "exec"), ns)
    out: dict = {}
    th = threading.Thread(target=ns["_thread_build"], args=(out, T, NCORES))
    th.start()
    th.join()
    if "err" in out:
        raise out["err"]
    return out["nc"]


_CACHE: dict = {}
_T, _B, _D, _H, _NC = 512, 64, 1024, 1024, 8


def _install_cc_cache():
    """Memoize the BIR->NEFF compile on the embedded BIR content.

    bass2jax recompiles the (unchanged) BIR on every jit trace; walrus takes
    ~0.4s per call even warm, and the outer HLO bytes differ per trace so an
    HLO-keyed cache never hits. The NEFF is a pure function of the BIR json
    and the io-rename maps, so cache the NEFF bytes keyed on those and only
    re-wrap them into each call's HLO. Any surprise falls back to the stock
    hook."""
    from concourse import bass2jax
    if getattr(bass2jax, "_convlstm_cc_cache", False):
        return
    import hashlib
    base_hook = bass2jax.neuronx_cc_hook
    cc_cache: dict = {}

    def _cached_hook(code, code_format, platform_version, file_prefix):
        try:
            code_b = bytes(code)
            if b"bass_exec" not in code_b:
                return base_hook(code, code_format, platform_version, file_prefix)
            import base64 as _b64
            import orjson as _oj
            import libneuronxla.proto.hlo_pb2 as _hp
            from libneuronxla.libncc import _wrap_neff_as_custom_call
            proto = _hp.HloModuleProto.FromString(code_b)
            call = None
            for comp in proto.computations:
                for ins in comp.instructions:
                    if (ins.opcode == "custom-call"
                            and ins.custom_call_target == "bass_exec"):
                        call = ins
            if call is None:
                return base_hook(code, code_format, platform_version, file_prefix)
            cfg = _oj.loads(_b64.standard_b64decode(call.backend_config))
            in_rename = {n: f"input{i}" for i, n in enumerate(cfg["in_names"])}
            out_rename = {n: f"output{i}" for i, n in enumerate(cfg["out_names"])}
            key = hashlib.blake2b(
                cfg["ant_bir"].encode()
                + _oj.dumps([in_rename, out_rename]),
                digest_size=16,
            ).digest()
            neff = cc_cache.get(key)
            if neff is None:
                import tempfile
                from concourse.bass_utils import compile_bir_kernel
                ant_bir = bass2jax._decompress_ant_bir(cfg["ant_bir"])
                nname = f"model_{proto.name.replace('/', '_')}.neff"
                with tempfile.TemporaryDirectory() as td:
                    nf = compile_bir_kernel(ant_bir, td, neff_name=nname)
                    neff = bass2jax.rename_neff_tensors_and_patch_header(
                        nf, in_rename | out_rename)
                if len(cc_cache) < 8:
                    cc_cache[key] = neff
            return 0, _wrap_neff_as_custom_call(code_b, neff)
        except Exception:
            return base_hook(code, code_format, platform_version, file_prefix)

    bass2jax.neuronx_cc_hook = _cached_hook
    bass2jax._convlstm_cc_cache = True


def _prep_w(Wfc, Wfc2):
    """Host-side weight prep -> per-core [16, 49152] bf16 row shards."""
    import numpy as np
    import ml_dtypes
    D, H, NC = _D, _H, _NC
    bf = ml_dtypes.bfloat16
    Wx = np.concatenate([Wfc[:H, :D].T, Wfc[H:, :D].T, Wfc2[:, :D].T], axis=1)   # [D, 3H]
    Wh = np.concatenate([Wfc[:H, D:].T, Wfc[H:, D:].T], axis=1)                  # [H, 2H]
    W2 = Wfc2[:, D:].T                                                           # [H, H]

    def kmaj(w):  # [K, M] -> [128, (K/128)*M]
        K, M = w.shape
        return np.ascontiguousarray(
            w.reshape(K // 128, 128, M).transpose(1, 0, 2).reshape(128, (K // 128) * M)
        ).astype(bf)

    w_all = np.concatenate([kmaj(Wx), kmaj(Wh), kmaj(W2)], axis=1)  # [128, 49152]
    rpc = 128 // NC
    return [np.ascontiguousarray(w_all[rpc * c:rpc * (c + 1)]) for c in range(NC)]


def _prep_x(x):
    """x [T,B,D] f32 -> per-core natural-layout bf16 chunks [TC*B, D]."""
    import ml_dtypes
    bf = ml_dtypes.bfloat16
    T, B, D = x.shape
    TC = T // _NC
    return [
        x[c * TC:(c + 1) * TC].reshape(TC * B, D).astype(bf) for c in range(_NC)
    ]


def _wfp(Wfc, Wfc2):
    """Cheap weight fingerprint: shapes + sums + sum-of-squares (memory-bound,
    ~10ms; detects any realistic weight change)."""
    import numpy as np
    a, b = Wfc.ravel(), Wfc2.ravel()
    return (
        Wfc.shape, Wfc2.shape,
        float(a.sum()), float(np.einsum("i,i->", a, a)),
        float(b.sum()), float(np.einsum("i,i->", b, b)),
    )


def kernel(**inputs):
    """Full-input kernel: x [512,64,1024] f32 -> h_seq [512,64,1024] f32."""
    import numpy as np
    x = np.asarray(inputs["x"], dtype=np.float32)
    Wfc = np.asarray(inputs["Wfc"], dtype=np.float32)
    Wfc2 = np.asarray(inputs["Wfc2"], dtype=np.float32)

    if "nc" not in _CACHE:
        _CACHE["nc"] = _build_nc(_T, _NC)
    _install_cc_cache()
    wkey = _wfp(Wfc, Wfc2)
    if _CACHE.get("wkey") != wkey:
        _CACHE["w"] = _prep_w(Wfc, Wfc2)
        _CACHE["wkey"] = wkey

    xs = _prep_x(x)
    in_maps = [{"xt": xs[c], "wsl": _CACHE["w"][c]} for c in range(_NC)]

    from concourse.bass_utils import run_bass_kernel_spmd
    try:
        res = run_bass_kernel_spmd(_CACHE["nc"], in_maps, list(range(_NC)))
    except Exception:
        # transient device/tunnel hiccups happen; one retry
        import time as _time
        _time.sleep(2.0)
        res = run_bass_kernel_spmd(_CACHE["nc"], in_maps, list(range(_NC)))
    out = np.empty((_T * _B, _H), dtype=np.float32)
    TB = _T * _B // _NC
    for c in range(_NC):
        out[c * TB:(c + 1) * TB] = res.results[c]["hout"]
    kernel.last_exec_time_ns = getattr(res, "exec_time_ns", None)
    return out.reshape(_T, _B, _H)
